# revision 9
# baseline (speedup 1.0000x reference)
"""BoundaryLoss kernel for 8 TRN2 NeuronCores.

Math (derived from the reference):
  - Sobel kernels have depth extent 1 -> depth slices independent; padded depth
    output slices are conv(0) = 0. sz == sy exactly, so
        loss_sum = sum(Gx^2) + 2*sum(Gy^2),
    with Gx = smooth_h[1,2,1] (x) diff_w[-1,0,1] applied to r,
         Gy = diff_h[-1,0,1] (x) smooth_w[1,2,1] applied to r,
         r  = softmax(pred, axis=C) - onehot(target)   ('same' zero padding).
  - Conv is linear: conv(p) - conv(t) = conv(p - t).

Implementation (per core; d-shard of 12 depth slices):
  layout: partitions = (c, h-chunk rows), free = (d, w); onehot precomputed
  host-side as uint8; pred cast to bf16 host-side (halves the dominant DMA).
  - exp on ScalarE (bf16 in/out); channel-sum replicated across the 4 c-groups
    via one bf16 TensorE matmul per d-triple with a 0/1 block lhsT;
  - p = e/s as a SINGLE DVE TensorTensor divide (psum f32 srep operand) --
    the cost model charges divide like mult, so this replaces the old
    reciprocal + normalize-mul pair at half the DVE time;
  - rp = oht - p on GpSimd (chunked per d-triple, balances DVE);
  - both 2D convs fully on TensorE in bf16 (FWL weight loads): banded
    block-diag lhsT for the h-direction factor, w-direction taps via shifted
    rhs/out APs accumulated in PSUM -- partial-coverage start=True clears
    has_written exactly where needed, giving exact 'same' zero-pad edges;
  - Square + free-dim reduce SPLIT between ScalarE (activation Square with
    accum_out; q=0,1) and DVE (native TensorTensorReduce mult+add; q=2,3),
    sqrt(2) baked into the Gy weights; per-(b,t,q) partial sums land in
    distinct columns of one [128, 48] accumulator (no cross-iter dep chains);
  - tc.high_priority() on the softmax stage so the Tile scheduler overlaps
    iteration i+1's softmax chain with iteration i's conv matmuls;
  - output: per-partition partial sums [128, 48]; host reduces + normalizes
    by B*(D+2)*(H+2)*(W+2)*C.
"""

import numpy as np
from contextlib import ExitStack

B, C, D, H, W = 2, 4, 96, 160, 160
NCORES = 8
DL = D // NCORES            # 12 depth slices per core
CH = 30                     # h-outputs per chunk
NT = 6                      # h-chunks (5*30 + 10)
NQ = 4                      # d-triples per (b, t)
DQ = DL // NQ               # 3
SQ2 = np.sqrt(2.0)

# per-chunk geometry: (in_start, in_rows, out_rows)
def _chunk_geom(t):
    out0 = CH * t
    outs = min(CH, H - out0)
    in0 = max(out0 - 1, 0)
    in1 = min(out0 + outs + 1, H)
    return in0, in1 - in0, outs


def _bands(t):
    """Banded matrices [rows, outs] for chunk t: (sh, dh) with 'same' padding."""
    in0, r, m = _chunk_geom(t)
    sh = np.zeros((r, m), np.float32)
    dh = np.zeros((r, m), np.float32)
    for mm in range(m):
        h_out = CH * t + mm
        for dlt, (cs, cd) in zip((-1, 0, 1), ((1.0, -1.0), (2.0, 0.0), (1.0, 1.0))):
            i = h_out + dlt - in0
            if 0 <= i < r:
                sh[i, mm] += cs
                dh[i, mm] += cd
    return sh, dh


def _blockdiag(b):
    r, m = b.shape
    out = np.zeros((4 * r, 4 * m), np.float32)
    for c in range(4):
        out[c * r:(c + 1) * r, c * m:(c + 1) * m] = b
    return out


def _build_consts():
    """Pack constants: bf16 [128, Y] (lsum + conv band matrices, sqrt2 baked
    into the Gy ones).

    Returns (cst_bf16, offs_b)."""
    import ml_dtypes
    colsb, offs_b, posb = [], {}, 0
    for t in range(NT):
        in0, r, m = _chunk_geom(t)
        p4 = 4 * r
        sh, dh = _bands(t)
        lsum = np.zeros((p4, p4), np.float32)
        for cp in range(4):
            for c in range(4):
                for i in range(r):
                    lsum[c * r + i, cp * r + i] = 1.0
        bufl = np.zeros((128, p4), ml_dtypes.bfloat16)
        bufl[:p4] = lsum.astype(ml_dtypes.bfloat16)
        colsb.append(bufl)
        offs_b[(t, "lsum")] = (posb, p4, p4)
        posb += p4
        mats = {
            "lshp": _blockdiag(sh),
            "lshm": _blockdiag(-sh),
            "ldh0": _blockdiag((2.0 * SQ2 * dh).astype(np.float32)),
            "ldh1": _blockdiag((SQ2 * dh).astype(np.float32)),
        }
        for name, mat in mats.items():
            rr, cc = mat.shape
            bufb = np.zeros((128, cc), ml_dtypes.bfloat16)
            bufb[:rr] = mat.astype(ml_dtypes.bfloat16)
            colsb.append(bufb)
            offs_b[(t, name)] = (posb, rr, cc)
            posb += cc
    return np.concatenate(colsb, axis=1), offs_b


def _build_nc(cstb_cols, offs_b, repeat=1):
    import concourse.bacc as bacc
    import concourse.tile as tile
    from concourse import mybir

    nc = bacc.Bacc()
    pred_d = nc.dram_tensor("pred", (B, C, H, DL, W), mybir.dt.bfloat16,
                            kind="ExternalInput")
    oh_d = nc.dram_tensor("oh", (B, C, H, DL, W), mybir.dt.bfloat16,
                          kind="ExternalInput")
    cstb_d = nc.dram_tensor("cstb", (128, cstb_cols), mybir.dt.bfloat16,
                            kind="ExternalInput")
    acc_d = nc.dram_tensor("acc", (128, B * NT * NQ), mybir.dt.float32,
                           kind="ExternalOutput")
    # bn_stats partials: 12 f32 per DVE-square slot (two 6-stat vectors)
    acc6_d = nc.dram_tensor("acc6", (128, B * NT * NQ * 12), mybir.dt.float32,
                            kind="ExternalOutput")

    with tile.TileContext(nc) as tc, ExitStack() as ctx:
        singles = ctx.enter_context(tc.tile_pool(name="singles", bufs=1))
        io = ctx.enter_context(tc.tile_pool(name="io", bufs=3))
        work = ctx.enter_context(tc.tile_pool(name="work", bufs=3))
        scr = ctx.enter_context(tc.tile_pool(name="scr", bufs=2))
        ps_s = ctx.enter_context(tc.tile_pool(name="ps_s", bufs=2, space="PSUM"))
        ps_c = ctx.enter_context(tc.tile_pool(name="ps_c", bufs=3, space="PSUM"))

        cstb = singles.tile([128, cstb_cols], mybir.dt.bfloat16)
        nc.sync.dma_start(out=cstb, in_=cstb_d[:, :])
        acc = singles.tile([128, B * NT * NQ], mybir.dt.float32)
        nc.vector.memset(acc, 0.0)
        acc6 = singles.tile([128, B * NT * NQ * 12], mybir.dt.float32)
        nc.vector.memset(acc6, 0.0)

        def lmatb(t, name):
            c0, rr, cc = offs_b[(t, name)]
            return cstb[:rr, c0:c0 + cc]

        def stage_a(b, t):
            """softmax: produce rp (bf16) = onehot - softmax(pred)."""
            in0, r, m = _chunk_geom(t)
            p4 = 4 * r
            raw = io.tile([128, DL, W], mybir.dt.bfloat16, tag="raw")
            oht = io.tile([128, DL, W], mybir.dt.bfloat16, tag="oht")
            nc.sync.dma_start(out=raw[0:p4, :, :],
                              in_=pred_d[b, :, in0:in0 + r, :, :])
            nc.sync.dma_start(out=oht[0:p4, :, :],
                              in_=oh_d[b, :, in0:in0 + r, :, :])
            e = work.tile([128, DL, W], mybir.dt.bfloat16, tag="e")
            nc.scalar.activation(e[:p4], raw[:p4],
                                 mybir.ActivationFunctionType.Exp)
            inv = work.tile([128, DL, W], mybir.dt.float32, tag="inv")
            for q in range(NQ):
                srep = ps_s.tile([128, DQ, W], mybir.dt.float32, tag="srep")
                nc.tensor.matmul(srep[:p4], lmatb(t, "lsum")[:p4, :p4],
                                 e[:p4, DQ * q:DQ * (q + 1), :],
                                 start=True, stop=True)
                nc.vector.reciprocal_approx_fast(
                    inv[:p4, DQ * q:DQ * (q + 1), :], srep[:p4])
            # normalize-mul on GpSimd (it is otherwise idle); subtract stays
            # on DVE where bf16 operands run in the 2x perf mode.
            p = work.tile([128, DL, W], mybir.dt.bfloat16, tag="p")
            rp = work.tile([128, DL, W], mybir.dt.bfloat16, tag="rp")
            half = DL // 2
            for h2 in range(2):
                sl = slice(half * h2, half * (h2 + 1))
                nc.gpsimd.tensor_mul(p[:p4, sl, :], e[:p4, sl, :],
                                     inv[:p4, sl, :])
                nc.vector.tensor_sub(rp[:p4, sl, :], oht[:p4, sl, :],
                                     p[:p4, sl, :])
            return rp

        def stage_b(b, t, rp):
            """conv + square-accumulate, TensorE-heavy, grouped by weight."""
            in0, r, m = _chunk_geom(t)
            p4, m4 = 4 * r, 4 * m
            shp, shm = lmatb(t, "lshp")[:p4, :m4], lmatb(t, "lshm")[:p4, :m4]
            dh0, dh1 = lmatb(t, "ldh0")[:p4, :m4], lmatb(t, "ldh1")[:p4, :m4]
            kw = dict(skip_group_check=True)
            convs, gxs, gys = [], [], []
            for q in range(NQ):
                conv = ps_c.tile([128, 2, 512], mybir.dt.float32, tag="conv")
                convs.append(conv)
                gxs.append(conv[:m4, 0, 0:DQ * W].rearrange(
                    "p (d w) -> p d w", w=W))
                gys.append(conv[:m4, 1, 0:DQ * W].rearrange(
                    "p (d w) -> p d w", w=W))
            rq = [rp[:p4, DQ * q:DQ * (q + 1), :] for q in range(NQ)]
            # per-q emission keeps each PSUM tile's lifetime short (6 mms + sq)
            for q in range(NQ):
                nc.tensor.matmul(gxs[q][:, :, W - 1:W], shm,
                                 rq[q][:, :, W - 2:W - 1],
                                 start=True, stop=False, **kw)
                nc.tensor.matmul(gxs[q][:, :, 0:W - 1], shp, rq[q][:, :, 1:W],
                                 start=True, stop=False, **kw)
                nc.tensor.matmul(gxs[q][:, :, 1:W - 1], shm, rq[q][:, :, 0:W - 2],
                                 start=False, stop=True, **kw)
                nc.tensor.matmul(gys[q][:, :, :], dh0, rq[q][:, :, :],
                                 start=True, stop=False, **kw)
                nc.tensor.matmul(gys[q][:, :, 0:W - 1], dh1, rq[q][:, :, 1:W],
                                 start=False, stop=False, **kw)
                nc.tensor.matmul(gys[q][:, :, 1:W], dh1, rq[q][:, :, 0:W - 1],
                                 start=False, stop=True, **kw)
                slot = (b * NT + t) * NQ + q
                # squares split between ScalarE (activation Square+accum) and
                # DVE (bn_stats per gx/gy row; host recombines c*(v+m^2)).
                # 2.8 : 1.2 per-iter average balances the two engines.
                n_act = 2 if (b * NT + t) % 5 == 4 else 3
                if q < n_act:
                    sqo = scr.tile([128, 2, DQ * W], mybir.dt.bfloat16,
                                   tag="sqo")
                    nc.scalar.activation(sqo[:m4], convs[q][:m4, :, 0:DQ * W],
                                         mybir.ActivationFunctionType.Square,
                                         accum_out=acc[:m4, slot:slot + 1])
                else:
                    c6 = slot * 12
                    nc.vector.bn_stats(acc6[:m4, c6:c6 + 6],
                                       convs[q][:m4, 0, 0:DQ * W])
                    nc.vector.bn_stats(acc6[:m4, c6 + 6:c6 + 12],
                                       convs[q][:m4, 1, 0:DQ * W])

        iters = [(b, t) for b in range(B) for t in range(NT)] * repeat
        skew = 1
        pending = []
        for (b, t) in iters:
            # high_priority: the scheduler eagerly runs the softmax chain the
            # moment deps clear, overlapping it with the previous iteration's
            # conv matmuls instead of queueing behind them.
            with tc.high_priority():
                rp = stage_a(b, t)
            pending.append((b, t, rp))
            if len(pending) > skew:
                stage_b(*pending.pop(0))
        for args in pending:
            stage_b(*args)

        nc.sync.dma_start(out=acc_d[:, :], in_=acc)
        nc.sync.dma_start(out=acc6_d[:, :], in_=acc6)

    if not nc.is_finalized():
        nc.finalize()
    return nc


LAST_RUNNER = None   # (callable, concat_inputs) for timing from test harnesses


def _make_runner(nc):
    """Compile nc into a reusable 8-core jitted callable.

    Mirrors bass2jax.run_bass_via_pjrt's multi-core tail, but without input
    donation so the callable can be invoked repeatedly for timing. Safe here
    because the single output ("acc") is fully written by the kernel's DMA.
    """
    import jax
    import numpy as _np
    from jax.sharding import Mesh, PartitionSpec
    from jax.experimental.shard_map import shard_map
    import concourse.mybir as mybir
    from concourse import bass2jax

    bass2jax.install_neuronx_cc_hook()

    pid_name = nc.partition_id_tensor.name if nc.partition_id_tensor else None
    in_names, out_names, out_avals = [], [], []
    for alloc in nc.m.functions[0].allocations:
        if not isinstance(alloc, mybir.MemoryLocationSet):
            continue
        name = alloc.memorylocations[0].name
        if alloc.kind == "ExternalInput":
            if name != pid_name:
                in_names.append(name)
        elif alloc.kind == "ExternalOutput":
            out_names.append(name)
            out_avals.append(jax.core.ShapedArray(
                tuple(alloc.tensor_shape), mybir.dt.np(alloc.dtype)))
    n_params = len(in_names)
    zero_outs = [_np.zeros(a.shape, a.dtype) for a in out_avals]
    all_names = in_names + out_names + ([pid_name] if pid_name else [])

    def _body(*args):
        operands = list(args)
        if pid_name is not None:
            operands.append(bass2jax.partition_id_tensor())
        outs = bass2jax._bass_exec_p.bind(
            *operands,
            out_avals=tuple(out_avals),
            in_names=tuple(all_names),
            out_names=tuple(out_names),
            lowering_input_output_aliases=(),
            sim_require_finite=True,
            sim_require_nnan=True,
            nc=nc,
        )
        return tuple(outs)

    devices = jax.devices()[:NCORES]
    mesh = Mesh(np.asarray(devices), ("core",))
    fn = jax.jit(shard_map(
        _body, mesh=mesh,
        in_specs=(PartitionSpec("core"),) * (n_params + len(out_names)),
        out_specs=(PartitionSpec("core"),) * len(out_names),
        check_rep=False), keep_unused=True)

    from jax.sharding import NamedSharding
    sh = NamedSharding(mesh, PartitionSpec("core"))
    cache = {}

    def run(in_maps):
        if "dev_in" not in cache:
            concat_in = [np.concatenate([m[nm] for m in in_maps], axis=0)
                         for nm in in_names]
            concat_zero = [np.zeros((NCORES * z.shape[0], *z.shape[1:]), z.dtype)
                           for z in zero_outs]
            cache["dev_in"] = [jax.device_put(a, sh) for a in concat_in]
            cache["dev_zero"] = [jax.device_put(a, sh) for a in concat_zero]
            jax.block_until_ready(cache["dev_in"])
        out = fn(*cache["dev_in"], *cache["dev_zero"])
        jax.block_until_ready(out)
        return {nm: np.asarray(out[i]) for i, nm in enumerate(out_names)}

    return run


def _prep_inputs(pred, target):
    import ml_dtypes
    pred = np.asarray(pred, dtype=np.float32)
    target = np.asarray(target)
    onehot = (target[:, None, :, :, :] == np.arange(C).reshape(1, C, 1, 1, 1)
              ).astype(np.uint8)                                 # (B,C,D,H,W)
    cstb, offs_b = _build_consts()
    in_maps = []
    for k in range(NCORES):
        sl = slice(k * DL, (k + 1) * DL)
        # (B,C,D,H,W) -> (B,C,H,DL,W) contiguous for fat DMA rows
        p_k = np.ascontiguousarray(
            pred[:, :, sl].transpose(0, 1, 3, 2, 4)).astype(ml_dtypes.bfloat16)
        o_k = np.ascontiguousarray(
            onehot[:, :, sl].transpose(0, 1, 3, 2, 4)).astype(ml_dtypes.bfloat16)
        in_maps.append({"pred": p_k, "oh": o_k, "cstb": cstb})
    return in_maps, (cstb, offs_b)


def kernel(pred, target):
    global LAST_RUNNER
    in_maps, (cstb, offs_b) = _prep_inputs(pred, target)
    nc = _build_nc(cstb.shape[1], offs_b)
    run = _make_runner(nc)
    LAST_RUNNER = (run, in_maps)

    # the axon terminal occasionally throws a transient device error on the
    # first execution after a NEFF switch; one retry has always cleared it
    try:
        outs = run(in_maps)
    except Exception:
        import time as _time
        _time.sleep(2.0)
        outs = run(in_maps)
    acc = outs["acc"]                              # (8*128, 48) concat
    st = outs["acc6"].astype(np.float64).reshape(-1, B * NT * NQ * 2, 6)
    # bn_stats layout per 6-vector: (cnt_even, mean_even, cnt*var_even,
    # cnt_odd, mean_odd, cnt*var_odd); sum of squares = cv + c*m^2 per half
    ss = (st[..., 2] + st[..., 0] * st[..., 1] ** 2
          + st[..., 5] + st[..., 3] * st[..., 4] ** 2)
    total = acc.astype(np.float64).sum() + ss.sum()
    per_tensor = B * (D + 2) * (H + 2) * (W + 2)
    loss = total / per_tensor / C
    return np.float32(loss)


# revision 28
# speedup vs baseline: 1.3788x; 1.3788x over previous
"""BoundaryLoss kernel for 8 TRN2 NeuronCores.

Math (derived from the reference):
  - Sobel kernels have depth extent 1 -> depth slices independent; padded depth
    output slices are conv(0) = 0. sz == sy exactly, so
        loss_sum = sum(Gx^2) + 2*sum(Gy^2),
    with Gx = smooth_h[1,2,1] (x) diff_w[-1,0,1] applied to r,
         Gy = diff_h[-1,0,1] (x) smooth_w[1,2,1] applied to r,
         r  = softmax(pred, axis=C) - onehot(target)   ('same' zero padding).
  - Conv is linear: conv(p) - conv(t) = conv(p - t).

Implementation (per core; d-shard of 12 depth slices):
  layout: partitions = (c, [group], h-chunk rows), free = (d, w). Engine cost
  in this regime is free-size-only, so a ragged tail chunk costs a FULL
  iteration; the two 10-row tail chunks (b=0, b=1) are merged into one
  88-partition iteration -> 11 pipeline iterations instead of 12.
  - pred and onehot cast to bf16 host-side (halves the dominant DMA);
  - exp on ScalarE (bf16 in/out); channel-sum replicated across the 4 c-groups
    via one bf16 TensorE matmul per d-triple with a 0/1 block lhsT;
  - reciprocal via the DVE RECIPROCAL_APPROX_FAST custom op (PSUM source);
  - p = e*inv on GpSimd (it is otherwise idle); rp = oht - p on DVE where the
    bf16 operands hit the 2x perf mode;
  - both 2D convs fully on TensorE in bf16 (FWL weight loads): banded
    block-diag lhsT for the h-direction factor, w-direction taps via shifted
    rhs/out APs accumulated in PSUM -- partial-coverage start=True clears
    has_written exactly where needed, giving exact 'same' zero-pad edges;
  - Square + free-dim reduce SPLIT between ScalarE (activation Square with
    accum_out) and DVE (bn_stats per gx/gy row; host recombines c*(v+m^2)),
    sqrt(2) baked into the Gy weights; per-(chunk,q) partial sums land in
    distinct columns of the accumulators (no cross-iter dep chains);
  - emission order per block: [exp/lsum/recip/mul (i)], [conv+squares (i-1)],
    [subs (i)] -- the previous iteration's squares sit ahead of this
    iteration's subs in the DVE stream, filling the wait for the GpSimd muls;
  - output: per-partition partial sums; host reduces + normalizes by
    B*(D+2)*(H+2)*(W+2)*C.
"""

import numpy as np
from contextlib import ExitStack

B, C, D, H, W = 2, 4, 96, 160, 160
NCORES = 8
DL = D // NCORES            # 12 depth slices per core
CH = 30                     # h-outputs per full chunk
NQ = 4                      # d-triples per chunk
DQ = DL // NQ               # 3
SQ2 = np.sqrt(2.0)


def _group_geom(out0, m):
    in0 = max(out0 - 1, 0)
    in1 = min(out0 + m + 1, H)
    return in0, in1 - in0


def _chunks():
    """List of chunks; each is a list of (b, out0, m, in0, r) groups.

    Full 30-row chunks are one group; the two 10-row tails share one chunk
    (4c x 2b x 11r = 88 partitions)."""
    ch = []
    for b in range(B):
        for t in range(5):
            out0, m = CH * t, CH
            in0, r = _group_geom(out0, m)
            ch.append([(b, out0, m, in0, r)])
    tail = []
    for b in range(B):
        out0, m = 150, 10
        in0, r = _group_geom(out0, m)
        tail.append((b, out0, m, in0, r))
    ch.append(tail)
    return ch


CHUNKS = _chunks()
NITER = len(CHUNKS)          # 11
NSLOT = NITER * NQ           # 44


def _band(out0, m, in0, r):
    """Banded matrices [r, m] for one group: (sh, dh) with 'same' padding."""
    sh = np.zeros((r, m), np.float32)
    dh = np.zeros((r, m), np.float32)
    for mm in range(m):
        h_out = out0 + mm
        for dlt, (cs, cd) in zip((-1, 0, 1), ((1.0, -1.0), (2.0, 0.0), (1.0, 1.0))):
            i = h_out + dlt - in0
            if 0 <= i < r:
                sh[i, mm] += cs
                dh[i, mm] += cd
    return sh, dh


def _chunk_pm(groups):
    R = sum(g[4] for g in groups)
    M = sum(g[2] for g in groups)
    return 4 * R, 4 * M, R, M


def _build_consts():
    """Pack constants: bf16 [128, Y] (lsum + conv band matrices, sqrt2 baked
    into the Gy ones). Block-diagonal over the (c, group) structure.

    Returns (cst_bf16, offs_b)."""
    import ml_dtypes
    colsb, offs_b, posb = [], {}, 0
    for ci, groups in enumerate(CHUNKS):
        p4, m4, R, M = _chunk_pm(groups)
        lsum = np.zeros((p4, p4), np.float32)
        goff = 0
        for (b, out0, m, in0, r) in groups:
            for i in range(r):
                for c in range(4):
                    for cp in range(4):
                        lsum[goff + c * r + i, goff + cp * r + i] = 1.0
            goff += 4 * r
        bufl = np.zeros((128, p4), ml_dtypes.bfloat16)
        bufl[:p4] = lsum.astype(ml_dtypes.bfloat16)
        colsb.append(bufl)
        offs_b[(ci, "lsum")] = (posb, p4, p4)
        posb += p4

        big = {k: np.zeros((p4, m4), np.float32)
               for k in ("lshp", "lshm", "ldh0", "ldh1")}
        goff, moff = 0, 0
        for (b, out0, m, in0, r) in groups:
            sh, dh = _band(out0, m, in0, r)
            mats = {"lshp": sh, "lshm": -sh,
                    "ldh0": 2.0 * SQ2 * dh, "ldh1": SQ2 * dh}
            for k, mat in mats.items():
                for c in range(4):
                    big[k][goff + c * r:goff + c * r + r,
                           moff + c * m:moff + c * m + m] = mat
            goff += 4 * r
            moff += 4 * m
        for k, mat in big.items():
            bufb = np.zeros((128, m4), ml_dtypes.bfloat16)
            bufb[:p4] = mat.astype(ml_dtypes.bfloat16)
            colsb.append(bufb)
            offs_b[(ci, k)] = (posb, p4, m4)
            posb += m4
    return np.concatenate(colsb, axis=1), offs_b


def _build_nc(cstb_cols, offs_b, repeat=1, skew=1, io_bufs=4, work_bufs=3,
              ps_c_bufs=3, hiprio=False, pf=2, ps_s_bufs=2,
              n_pool_sub=0, n_act_num=14, n_act_den=5):
    import concourse.bacc as bacc
    import concourse.tile as tile
    from concourse import mybir

    nc = bacc.Bacc()
    pred_d = nc.dram_tensor("pred", (B, C, H, DL, W), mybir.dt.bfloat16,
                            kind="ExternalInput")
    oh_d = nc.dram_tensor("oh", (B, C, H, DL, W), mybir.dt.bfloat16,
                          kind="ExternalInput")
    cstb_d = nc.dram_tensor("cstb", (128, cstb_cols), mybir.dt.bfloat16,
                            kind="ExternalInput")
    acc_d = nc.dram_tensor("acc", (128, NSLOT), mybir.dt.float32,
                           kind="ExternalOutput")
    # bn_stats partials: 12 f32 per DVE-square slot (two 6-stat vectors)
    acc6_d = nc.dram_tensor("acc6", (128, NSLOT * 12), mybir.dt.float32,
                            kind="ExternalOutput")

    with tile.TileContext(nc) as tc, ExitStack() as ctx:
        singles = ctx.enter_context(tc.tile_pool(name="singles", bufs=1))
        io = ctx.enter_context(tc.tile_pool(name="io", bufs=io_bufs))
        work = ctx.enter_context(tc.tile_pool(name="work", bufs=work_bufs))
        scr = ctx.enter_context(tc.tile_pool(name="scr", bufs=2))
        ps_s = ctx.enter_context(tc.tile_pool(name="ps_s", bufs=ps_s_bufs,
                                              space="PSUM"))
        ps_c = ctx.enter_context(tc.tile_pool(name="ps_c", bufs=ps_c_bufs,
                                              space="PSUM"))

        cstb = singles.tile([128, cstb_cols], mybir.dt.bfloat16)
        nc.sync.dma_start(out=cstb, in_=cstb_d[:, :])
        acc = singles.tile([128, NSLOT], mybir.dt.float32)
        nc.vector.memset(acc, 0.0)
        acc6 = singles.tile([128, NSLOT * 12], mybir.dt.float32)
        nc.vector.memset(acc6, 0.0)

        def lmatb(ci, name):
            c0, rr, cc = offs_b[(ci, name)]
            return cstb[:rr, c0:c0 + cc]

        def stage_dma(ci):
            """input DMAs, issued PF iterations ahead of the compute."""
            groups = CHUNKS[ci]
            p4, m4, R, M = _chunk_pm(groups)
            raw = io.tile([128, DL, W], mybir.dt.bfloat16, tag="raw")
            oht = io.tile([128, DL, W], mybir.dt.bfloat16, tag="oht")
            goff = 0
            for (b, out0, m, in0, r) in groups:
                nc.sync.dma_start(out=raw[goff:goff + 4 * r, :, :],
                                  in_=pred_d[b, :, in0:in0 + r, :, :])
                nc.sync.dma_start(out=oht[goff:goff + 4 * r, :, :],
                                  in_=oh_d[b, :, in0:in0 + r, :, :])
                goff += 4 * r
            return raw, oht

        def stage_a(ci, raw, oht):
            """softmax: produce rp (bf16) = onehot - softmax(pred)."""
            p4, m4, R, M = _chunk_pm(CHUNKS[ci])
            e = work.tile([128, DL, W], mybir.dt.bfloat16, tag="e")
            nc.scalar.activation(e[:p4], raw[:p4],
                                 mybir.ActivationFunctionType.Exp)
            inv = work.tile([128, DL, W], mybir.dt.float32, tag="inv")
            p = work.tile([128, DL, W], mybir.dt.bfloat16, tag="p")
            rp = work.tile([128, DL, W], mybir.dt.bfloat16, tag="rp")
            for q in range(NQ):
                sl = slice(DQ * q, DQ * (q + 1))
                srep = ps_s.tile([128, DQ, W], mybir.dt.float32, tag="srep")
                nc.tensor.matmul(srep[:p4], lmatb(ci, "lsum")[:p4, :p4],
                                 e[:p4, sl, :], start=True, stop=True)
                nc.vector.reciprocal_approx_fast(
                    inv[:p4, sl, :], srep[:p4])
                # normalize-mul on GpSimd (it is otherwise idle)
                nc.gpsimd.tensor_mul(p[:p4, sl, :], e[:p4, sl, :],
                                     inv[:p4, sl, :])

            def subs():
                for q in range(NQ):
                    sl = slice(DQ * q, DQ * (q + 1))
                    if q < n_pool_sub:
                        nc.gpsimd.tensor_sub(rp[:p4, sl, :], oht[:p4, sl, :],
                                             p[:p4, sl, :])
                    else:
                        nc.vector.tensor_sub(rp[:p4, sl, :], oht[:p4, sl, :],
                                             p[:p4, sl, :])
            return rp, subs

        def stage_b(ci, rp):
            """conv + square-accumulate, TensorE-heavy, grouped by weight."""
            p4, m4, R, M = _chunk_pm(CHUNKS[ci])
            shp, shm = lmatb(ci, "lshp")[:p4, :m4], lmatb(ci, "lshm")[:p4, :m4]
            dh0, dh1 = lmatb(ci, "ldh0")[:p4, :m4], lmatb(ci, "ldh1")[:p4, :m4]
            kw = dict(skip_group_check=True)
            n_act = ((ci + 1) * n_act_num // n_act_den
                     - ci * n_act_num // n_act_den)
            # per-q emission keeps each PSUM tile's lifetime short (6 mms + sq)
            for q in range(NQ):
                conv = ps_c.tile([128, 2, 512], mybir.dt.float32, tag="conv")
                gx = conv[:m4, 0, 0:DQ * W].rearrange("p (d w) -> p d w", w=W)
                gy = conv[:m4, 1, 0:DQ * W].rearrange("p (d w) -> p d w", w=W)
                rq = rp[:p4, DQ * q:DQ * (q + 1), :]
                nc.tensor.matmul(gx[:, :, W - 1:W], shm, rq[:, :, W - 2:W - 1],
                                 start=True, stop=False, **kw)
                nc.tensor.matmul(gx[:, :, 0:W - 1], shp, rq[:, :, 1:W],
                                 start=True, stop=False, **kw)
                nc.tensor.matmul(gx[:, :, 1:W - 1], shm, rq[:, :, 0:W - 2],
                                 start=False, stop=True, **kw)
                nc.tensor.matmul(gy[:, :, :], dh0, rq[:, :, :],
                                 start=True, stop=False, **kw)
                nc.tensor.matmul(gy[:, :, 0:W - 1], dh1, rq[:, :, 1:W],
                                 start=False, stop=False, **kw)
                nc.tensor.matmul(gy[:, :, 1:W], dh1, rq[:, :, 0:W - 1],
                                 start=False, stop=True, **kw)
                slot = ci * NQ + q
                # squares split between ScalarE (activation Square+accum) and
                # DVE (bn_stats per gx/gy row; host recombines c*(v+m^2))
                if q < n_act:
                    sqo = scr.tile([128, 2, DQ * W], mybir.dt.bfloat16,
                                   tag="sqo")
                    nc.scalar.activation(sqo[:m4], conv[:m4, :, 0:DQ * W],
                                         mybir.ActivationFunctionType.Square,
                                         accum_out=acc[:m4, slot:slot + 1])
                else:
                    c6 = slot * 12
                    nc.vector.bn_stats(acc6[:m4, c6:c6 + 6],
                                       conv[:m4, 0, 0:DQ * W])
                    nc.vector.bn_stats(acc6[:m4, c6 + 6:c6 + 12],
                                       conv[:m4, 1, 0:DQ * W])

        iters = list(range(NITER)) * repeat
        pending = []
        dmas = []
        for i in range(min(pf, len(iters))):
            dmas.append(stage_dma(iters[i]))
        for i, ci in enumerate(iters):
            if i + pf < len(iters):
                dmas.append(stage_dma(iters[i + pf]))
            raw, oht = dmas.pop(0)
            # emission order per block: [exp/lsum/recip/mul (i)],
            # [conv+squares (i-1)], [subs (i)] -- puts the previous
            # iteration's squares ahead of this iteration's subs in the DVE
            # stream, where they fill the wait for the GpSimd mul chain.
            if hiprio:
                with tc.high_priority():
                    rp, subs = stage_a(ci, raw, oht)
            else:
                rp, subs = stage_a(ci, raw, oht)
            if len(pending) >= skew:
                stage_b(*pending.pop(0))
            subs()
            pending.append((ci, rp))
        for args in pending:
            stage_b(*args)

        nc.sync.dma_start(out=acc_d[:, :], in_=acc)
        nc.sync.dma_start(out=acc6_d[:, :], in_=acc6)

    if not nc.is_finalized():
        nc.finalize()
    return nc


LAST_RUNNER = None   # (callable, concat_inputs) for timing from test harnesses


def _make_runner(nc):
    """Compile nc into a reusable 8-core jitted callable.

    Mirrors bass2jax.run_bass_via_pjrt's multi-core tail, but without input
    donation so the callable can be invoked repeatedly for timing. Safe here
    because the outputs are fully written by the kernel's DMAs.
    """
    import jax
    import numpy as _np
    from jax.sharding import Mesh, PartitionSpec
    from jax.experimental.shard_map import shard_map
    import concourse.mybir as mybir
    from concourse import bass2jax

    bass2jax.install_neuronx_cc_hook()

    pid_name = nc.partition_id_tensor.name if nc.partition_id_tensor else None
    in_names, out_names, out_avals = [], [], []
    for alloc in nc.m.functions[0].allocations:
        if not isinstance(alloc, mybir.MemoryLocationSet):
            continue
        name = alloc.memorylocations[0].name
        if alloc.kind == "ExternalInput":
            if name != pid_name:
                in_names.append(name)
        elif alloc.kind == "ExternalOutput":
            out_names.append(name)
            out_avals.append(jax.core.ShapedArray(
                tuple(alloc.tensor_shape), mybir.dt.np(alloc.dtype)))
    n_params = len(in_names)
    zero_outs = [_np.zeros(a.shape, a.dtype) for a in out_avals]
    all_names = in_names + out_names + ([pid_name] if pid_name else [])

    def _body(*args):
        operands = list(args)
        if pid_name is not None:
            operands.append(bass2jax.partition_id_tensor())
        outs = bass2jax._bass_exec_p.bind(
            *operands,
            out_avals=tuple(out_avals),
            in_names=tuple(all_names),
            out_names=tuple(out_names),
            lowering_input_output_aliases=(),
            sim_require_finite=True,
            sim_require_nnan=True,
            nc=nc,
        )
        return tuple(outs)

    devices = jax.devices()[:NCORES]
    mesh = Mesh(np.asarray(devices), ("core",))
    fn = jax.jit(shard_map(
        _body, mesh=mesh,
        in_specs=(PartitionSpec("core"),) * (n_params + len(out_names)),
        out_specs=(PartitionSpec("core"),) * len(out_names),
        check_rep=False), keep_unused=True)

    from jax.sharding import NamedSharding
    sh = NamedSharding(mesh, PartitionSpec("core"))
    cache = {}

    def run(in_maps):
        if "dev_in" not in cache:
            concat_in = [np.concatenate([m[nm] for m in in_maps], axis=0)
                         for nm in in_names]
            concat_zero = [np.zeros((NCORES * z.shape[0], *z.shape[1:]), z.dtype)
                           for z in zero_outs]
            cache["dev_in"] = [jax.device_put(a, sh) for a in concat_in]
            cache["dev_zero"] = [jax.device_put(a, sh) for a in concat_zero]
            jax.block_until_ready(cache["dev_in"])
        out = fn(*cache["dev_in"], *cache["dev_zero"])
        jax.block_until_ready(out)
        return {nm: np.asarray(out[i]) for i, nm in enumerate(out_names)}

    return run


def _prep_inputs(pred, target):
    import ml_dtypes
    pred = np.asarray(pred, dtype=np.float32)
    target = np.asarray(target)
    onehot = (target[:, None, :, :, :] == np.arange(C).reshape(1, C, 1, 1, 1)
              ).astype(np.float32)                               # (B,C,D,H,W)
    cstb, offs_b = _build_consts()
    in_maps = []
    for k in range(NCORES):
        sl = slice(k * DL, (k + 1) * DL)
        # (B,C,D,H,W) -> (B,C,H,DL,W) contiguous for fat DMA rows
        p_k = np.ascontiguousarray(
            pred[:, :, sl].transpose(0, 1, 3, 2, 4)).astype(ml_dtypes.bfloat16)
        o_k = np.ascontiguousarray(
            onehot[:, :, sl].transpose(0, 1, 3, 2, 4)).astype(ml_dtypes.bfloat16)
        in_maps.append({"pred": p_k, "oh": o_k, "cstb": cstb})
    return in_maps, (cstb, offs_b)


def kernel(pred, target):
    global LAST_RUNNER
    in_maps, (cstb, offs_b) = _prep_inputs(pred, target)
    nc = _build_nc(cstb.shape[1], offs_b)
    run = _make_runner(nc)
    LAST_RUNNER = (run, in_maps)

    # the axon terminal occasionally throws a transient device error on the
    # first execution after a NEFF switch; one retry has always cleared it
    try:
        outs = run(in_maps)
    except Exception:
        import time as _time
        _time.sleep(2.0)
        outs = run(in_maps)
    acc = outs["acc"]                              # (8*128, 44) concat
    st = outs["acc6"].astype(np.float64).reshape(-1, NSLOT * 2, 6)
    # bn_stats layout per 6-vector: (cnt_even, mean_even, cnt*var_even,
    # cnt_odd, mean_odd, cnt*var_odd); sum of squares = cv + c*m^2 per half
    ss = (st[..., 2] + st[..., 0] * st[..., 1] ** 2
          + st[..., 5] + st[..., 3] * st[..., 4] ** 2)
    total = acc.astype(np.float64).sum() + ss.sum()
    per_tensor = B * (D + 2) * (H + 2) * (W + 2)
    loss = total / per_tensor / C
    return np.float32(loss)
